# revision 7
# baseline (speedup 1.0000x reference)
"""TRN2 Bass kernel for nn_DynamicWeightProjection (8-core data parallel).

Math (per token row of x = query_vec [B*T, D]):
    h_c  = gelu_exact(x @ dw1[:,0,c,:])  for the two live splits c in {0,2}
    w_c  = h_c @ qkw[0,c]                 ([128]->[4,32], rms-normed on host)
    ddv  = tanh(x @ dd[:,0,cols])         cols {0:32, 64:96}
Output = [rms(w_0 i01), rms(w_0 i23)*s, ddv[:32], rms(w_2 i01),
          rms(w_2 i23)*s, ddv[32:64]]  (320 cols, fp32)

Device design (weights-stationary mm1):
  - 8-way data parallel over rows: 2048 rows/core, processed in 4
    quarters of 512 rows (one PSUM bank per fp32 [128,512] accumulator).
  - mm1 keeps the dw1/dd weight chunks STATIONARY and streams xT chunks
    as the moving operand, producing hT = [k, rows] directly -- the
    layout mm2 needs, so no PE transposes anywhere (v1 spent ~9us/rep
    on 32 transposes plus 512 LDWEIGHTS of streamed x chunks).
  - The 64-wide dd weights are split into 32-col stationaries and
    col-tiled 4-way at tile_position (0,0/32/64/96) into four PSUM
    banks. PE tile packing only runs tiles concurrently when each
    stationary fits a single 32-wide col-group: 2-way 64-col tiles
    executed serially (measured +7us/rep median vs this layout). The
    partial sums are added on the host (tanh is host-side too).
  - mm2 keeps qkw stationary and streams gelu(hT) as a single N=512
    moving operand per split (2 MMs/quarter instead of 8 short ones);
    w lands transposed [im, rows] and the host epilogue untransposes.
    It is software-pipelined into the next quarter's mm1 stream.
  - The rms normalization, norm_scale, tanh and final column assembly
    run on the host: the device then only ever uses Gelu/Copy on the
    ACT engine (one activation-table set -- no ~2.7us table reloads)
    and the DVE does nothing.
  - bf16 everywhere on device (inputs, outputs); f32 only in PSUM.
    Measured rel err ~5.0e-3 vs the fp32 reference (gate 2e-2).
"""
import numpy as np
from contextlib import ExitStack

import ml_dtypes

import concourse.bacc as bacc
import concourse.mybir as mybir
import concourse.tile as tile
from concourse.bass_utils import run_bass_kernel_spmd

AF = mybir.ActivationFunctionType
F32 = mybir.dt.float32
BF16 = mybir.dt.bfloat16

B, T, D = 4, 4096, 4096
NCORES = 8
ROWS = (B * T) // NCORES        # 2048 rows per core
NQ = 4                          # quarters per core
QR = ROWS // NQ                 # 512 rows per quarter
DC = D // 128                   # 32 contraction chunks
GRP = 4                         # x chunks per DMA tile
NGRP = DC // GRP                # 8 groups
EPS = 1.1920929e-07


def build_nc(repeat=1, variant="full"):
    """variant: "full" = real kernel; "mm1"/"noout" are timing ablations."""
    nc = bacc.Bacc("TRN2", target_bir_lowering=False, debug=False,
                   num_devices=NCORES, enable_partition_id=False)

    # [q, g, p, j, r]: each x tile (4 chunks x 512 rows) is one fully
    # contiguous 4 KiB-per-partition DMA
    xq_in = nc.dram_tensor("xq", [NQ, NGRP, 128, GRP, QR], BF16,
                           kind="ExternalInput")
    wc_in = nc.dram_tensor("wc", [128, DC, 256], BF16, kind="ExternalInput")
    wdd_in = nc.dram_tensor("wdd", [128, DC, 64], BF16, kind="ExternalInput")
    qkw_in = nc.dram_tensor("qkw2", [128, 2, 128], BF16, kind="ExternalInput")
    # raw (pre-rms) w, transposed [c, q, im, r]: mm2 keeps qkw stationary
    # and streams g as one N=512 moving operand (2 MMs/quarter instead of
    # 8 short ones); the host epilogue untransposes
    out_main = nc.dram_tensor("out_main", [2, NQ, 128, QR], BF16,
                              kind="ExternalOutput")
    out_ddraw = nc.dram_tensor("out_ddraw", [128, ROWS], BF16,
                               kind="ExternalOutput")

    NQTOT = repeat * NQ

    with tile.TileContext(nc) as tc, ExitStack() as ctx:
        consts = ctx.enter_context(tc.tile_pool(name="consts", bufs=1))
        xpool = ctx.enter_context(tc.tile_pool(name="x", bufs=16))
        gpool = ctx.enter_context(tc.tile_pool(name="g", bufs=4))
        ddpool = ctx.enter_context(tc.tile_pool(name="dds", bufs=2))
        wpool = ctx.enter_context(tc.tile_pool(name="w", bufs=4))
        # PSUM (8 banks total): ppool = double-buffered hc accumulators
        # (2/quarter x 2 quarters); p2pool = shared 4-bank ring for the 4
        # dd col-tile accumulators + 2 mm2 outputs (6 allocs/quarter).
        # dd tiles are allocated at chunk-group 2 and first written at
        # group 3, by which time the previous quarter's ddcopy/mm2-copy
        # ACT reads have freed the aliased banks.
        ppool = ctx.enter_context(tc.tile_pool(name="ps", bufs=4, space="PSUM"))
        p2pool = ctx.enter_context(tc.tile_pool(name="ps2", bufs=4,
                                                space="PSUM"))

        wc_sb = consts.tile([128, DC, 256], BF16)
        wdd_sb = consts.tile([128, DC, 64], BF16)
        qkw_sb = consts.tile([128, 2, 128], BF16)

        # Prologue: weights interleaved with Q0's x tiles in consumption
        # order so the first matmuls don't wait on the full 2.6 MiB.
        q0_tiles = []
        for g in range(NGRP):
            nc.sync.dma_start(wc_sb[:, g * GRP:(g + 1) * GRP, :],
                              wc_in[:, g * GRP:(g + 1) * GRP, :])
            nc.sync.dma_start(wdd_sb[:, g * GRP:(g + 1) * GRP, :],
                              wdd_in[:, g * GRP:(g + 1) * GRP, :])
            xt = xpool.tile([128, GRP, QR], BF16, tag="xt")
            nc.sync.dma_start(xt[:], xq_in[0, g])
            q0_tiles.append(xt)
        nc.sync.dma_start(qkw_sb[:], qkw_in[:])

        def emit_mm1(Q, tiles, tail_prev, next_tiles_out):
            """mm1 for quarter Q; emits tail_prev() after group 1 and
            prefetch DMAs for Q+1 spread across the groups."""
            hc0 = ppool.tile([128, QR], F32, tag="ps")
            hc2 = ppool.tile([128, QR], F32, tag="ps")
            dd4t = []

            def dd_pairs(g):
                # 4-way col-tiling with 32-col (single col-group)
                # stationaries at (0,0/32/64/96) into 4 banks: 64-col
                # 2-way tiles never packed concurrently (measured), but
                # single-col-group tiles do (-7us/rep). Bank t holds
                # partitions [32t:32t+32]: [even-lo; even-hi; odd-lo;
                # odd-hi], so raw[0:64]+raw[64:128] on the host is still
                # [pre_total; post_total].
                xt = tiles[g]
                d0 = g * GRP
                for j in range(0, GRP, 2):
                    for t in range(4):
                        dj = j + t // 2
                        lo = (t % 2) * 32
                        nc.tensor.matmul(
                            dd4t[t][t * 32:(t + 1) * 32, :],
                            wdd_sb[:, d0 + dj, lo:lo + 32],
                            xt[:, dj, :], start=d0 + j == 0,
                            stop=d0 + j == DC - 2,
                            tile_position=(0, t * 32))

            for g in range(NGRP):
                xt = tiles[g]
                d0 = g * GRP
                # runs of GRP MMs per psum bank to limit bank cycling
                for j in range(GRP):
                    nc.tensor.matmul(hc0[:], wc_sb[:, d0 + j, 0:128],
                                     xt[:, j, :], start=d0 + j == 0,
                                     stop=d0 + j == DC - 1)
                for j in range(GRP):
                    nc.tensor.matmul(hc2[:], wc_sb[:, d0 + j, 128:256],
                                     xt[:, j, :], start=d0 + j == 0,
                                     stop=d0 + j == DC - 1)
                if g == 2:
                    # dd banks allocated late: they reuse ddcopy/mm2-freed
                    # banks of the 4-ring; first dd write is at g==3
                    for t in range(4):
                        dd4t.append(p2pool.tile([128, QR], F32,
                                                name=f"dd4_{t}", tag="ps2"))
                if g >= 3:
                    dd_pairs(g - 3)
                if g == 1 and tail_prev is not None:
                    tail_prev()
                if Q + 1 < NQTOT:
                    xt1 = xpool.tile([128, GRP, QR], BF16, tag="xt")
                    nc.sync.dma_start(xt1[:], xq_in[(Q + 1) % NQ, g])
                    next_tiles_out.append(xt1)
            for g in range(NGRP - 3, NGRP):
                dd_pairs(g)
            return hc0, hc2, dd4t

        def emit_head_tail(Q, hc0, hc2, dd4t):
            """ACT work right after mm1(Q): gelu + dd evacuation."""
            g_sb = gpool.tile([128, 2, QR], BF16, tag="g")
            nc.scalar.activation(g_sb[:, 0, :], hc0[:], AF.Gelu)
            nc.scalar.activation(g_sb[:, 1, :], hc2[:], AF.Gelu)
            dds = ddpool.tile([128, QR], BF16, tag="dds")
            for t in range(4):
                nc.scalar.activation(dds[t * 32:(t + 1) * 32, :],
                                     dd4t[t][t * 32:(t + 1) * 32, :], AF.Copy)
            return g_sb, dds

        def make_tail(Q, g_sb, dds):
            """mm2 + raw-w store for quarter Q (run during Q+1)."""
            q = Q % NQ

            def tail():
                if variant == "noout":
                    pass
                else:
                    nc.scalar.dma_start(out_ddraw[:, q * QR:(q + 1) * QR],
                                        dds[:])
                for c in range(2):
                    m2 = p2pool.tile([128, QR], F32, tag="ps2")
                    nc.tensor.matmul(m2[:], qkw_sb[:, c, :], g_sb[:, c, :],
                                     start=True, stop=True)
                    w = wpool.tile([128, QR], BF16, tag="wsb")
                    nc.scalar.activation(w[:], m2[:], AF.Copy)
                    if variant != "noout":
                        nc.scalar.dma_start(out_main[c, q], w[:])

            return tail

        tiles = q0_tiles
        tail_prev = None
        for Q in range(NQTOT):
            next_tiles = []
            hc0, hc2, dd4t = emit_mm1(Q, tiles, tail_prev, next_tiles)
            g_sb, dds = emit_head_tail(Q, hc0, hc2, dd4t)
            tail_prev = None if variant == "mm1" else make_tail(Q, g_sb, dds)
            tiles = next_tiles
        if tail_prev is not None:
            tail_prev()

    nc.compile()
    return nc


def host_prep(query_vec, dw1, qkw, dd, norm_scale):
    """Per-core input maps, all bf16."""
    x = np.ascontiguousarray(query_vec.reshape(B * T, D)).astype(
        ml_dtypes.bfloat16)

    wsel = dw1[:, 0][:, [0, 2], :].reshape(D, 256)
    wc_h = np.ascontiguousarray(
        wsel.reshape(DC, 128, 256).transpose(1, 0, 2)).astype(
        ml_dtypes.bfloat16)                                    # [128, DC, 256]
    ddsel = np.concatenate([dd[:, 0, 0:32], dd[:, 0, 64:96]], axis=1)
    wdd_h = np.ascontiguousarray(
        ddsel.reshape(DC, 128, 64).transpose(1, 0, 2)).astype(
        ml_dtypes.bfloat16)                                    # [128, DC, 64]
    qkw2 = np.ascontiguousarray(
        qkw[0, [0, 2]].reshape(2, 128, 128).transpose(1, 0, 2)
    ).astype(ml_dtypes.bfloat16)                               # [128, 2, 128]

    in_maps = []
    for c in range(NCORES):
        xc = x[c * ROWS:(c + 1) * ROWS]                        # [2048, 4096]
        xh = np.ascontiguousarray(
            xc.reshape(NQ, QR, NGRP, GRP, 128).transpose(0, 2, 4, 3, 1))
        in_maps.append({"xq": xh, "wc": wc_h, "wdd": wdd_h, "qkw2": qkw2})
    return in_maps


def host_post(results, norm_scale):
    """rms-normalize raw w, finish dd (sum halves + tanh), assemble."""
    s = float(np.asarray(norm_scale).reshape(-1)[0])
    outs = []
    for c in range(NCORES):
        om = np.asarray(results[c]["out_main"], dtype=np.float32)
        # [c, q, im, r] -> [q, r, c, im] -> [rows, c, i, m]
        w = om.transpose(1, 3, 0, 2).reshape(ROWS, 2, 4, 32)
        rms = np.sqrt(np.mean(w * w, axis=-1, keepdims=True) + EPS)
        w = w / rms
        w[:, :, 2:4] *= s
        w = w.reshape(ROWS, 2, 128)
        ddraw = np.asarray(results[c]["out_ddraw"], dtype=np.float32)
        ddv = np.tanh(ddraw[0:64] + ddraw[64:128]).T           # [2048, 64]
        outs.append(np.concatenate(
            [w[:, 0], ddv[:, 0:32], w[:, 1], ddv[:, 32:64]], axis=1))
    return np.concatenate(outs, axis=0).reshape(B, T, 320)


_NC_CACHE = {}


def get_nc():
    if "nc" not in _NC_CACHE:
        _NC_CACHE["nc"] = build_nc()
    return _NC_CACHE["nc"]


def _run_device(nc, in_maps):
    return run_bass_kernel_spmd(nc, in_maps, list(range(NCORES))).results


def _run_subprocess(query_vec, dw1, qkw, dd, norm_scale):
    """Fresh-process fallback: a crashed/wedged device state lives in the
    axon client; a clean process (with core reset) usually recovers."""
    import os
    import subprocess
    import sys
    import tempfile
    d = tempfile.mkdtemp(prefix="dwp_kernel_")
    np.save(os.path.join(d, "query_vec.npy"), query_vec)
    np.save(os.path.join(d, "dw1.npy"), dw1)
    np.save(os.path.join(d, "qkw.npy"), qkw)
    np.save(os.path.join(d, "dd.npy"), dd)
    np.save(os.path.join(d, "norm_scale.npy"), norm_scale)
    prog = (
        "import numpy as np, importlib.util, sys\n"
        f"spec = importlib.util.spec_from_file_location('dwp_kernel', {__file__!r})\n"
        "m = importlib.util.module_from_spec(spec); spec.loader.exec_module(m)\n"
        f"d = {d!r}\n"
        "ins = {k: np.load(d + '/' + k + '.npy') for k in"
        " ('query_vec', 'dw1', 'qkw', 'dd', 'norm_scale')}\n"
        "out = m.kernel(_allow_subprocess=False, **ins)\n"
        "np.save(d + '/out.npy', out)\n"
    )
    env = dict(os.environ)
    env["NEURON_RT_RESET_CORES"] = "1"
    subprocess.run([sys.executable, "-c", prog], check=True, env=env,
                   timeout=1800)
    return np.load(os.path.join(d, "out.npy"))


def kernel(query_vec, dw1, qkw, dd, norm_scale, _allow_subprocess=True):
    nc = get_nc()
    in_maps = host_prep(query_vec, dw1, qkw, dd, norm_scale)
    try:
        res = _run_device(nc, in_maps)
    except Exception:
        if not _allow_subprocess:
            raise
        try:
            res = _run_device(nc, in_maps)       # in-process retry
        except Exception:
            return _run_subprocess(query_vec, dw1, qkw, dd, norm_scale)
    return host_post(res, norm_scale)


# revision 9
# speedup vs baseline: 1.0254x; 1.0254x over previous
"""TRN2 Bass kernel for nn_DynamicWeightProjection (8-core data parallel).

Math (per token row of x = query_vec [B*T, D]):
    h_c  = gelu_exact(x @ dw1[:,0,c,:])  for the two live splits c in {0,2}
    w_c  = h_c @ qkw[0,c]                 ([128]->[4,32], rms-normed on host)
    ddv  = tanh(x @ dd[:,0,cols])         cols {0:32, 64:96}
Output = [rms(w_0 i01), rms(w_0 i23)*s, ddv[:32], rms(w_2 i01),
          rms(w_2 i23)*s, ddv[32:64]]  (320 cols, fp32)

Device design (weights-stationary mm1):
  - 8-way data parallel over rows: 2048 rows/core, processed in 4
    quarters of 512 rows (one PSUM bank per fp32 [128,512] accumulator).
  - mm1 keeps the dw1/dd weight chunks STATIONARY and streams xT chunks
    as the moving operand, producing hT = [k, rows] directly -- the
    layout mm2 needs, so no PE transposes anywhere (v1 spent ~9us/rep
    on 32 transposes plus 512 LDWEIGHTS of streamed x chunks).
  - The 64-wide dd weights are split into 32-col stationaries and
    col-tiled 4-way at tile_position (0,0/32/64/96) into four PSUM
    banks. PE tile packing only runs tiles concurrently when each
    stationary fits a single 32-wide col-group: 2-way 64-col tiles
    executed serially (measured +7us/rep median vs this layout). The
    partial sums are added on the host (tanh is host-side too).
  - mm2 keeps qkw stationary and streams gelu(hT) as a single N=512
    moving operand per split (2 MMs/quarter instead of 8 short ones);
    w lands transposed [im, rows] and the host epilogue untransposes.
    It is software-pipelined into the next quarter's mm1 stream.
  - The rms normalization, norm_scale, tanh and final column assembly
    run on the host: the device then only ever uses Gelu/Copy on the
    ACT engine (one activation-table set -- no ~2.7us table reloads);
    the DVE only evacuates the dd accumulators, overlapping the gelus.
  - bf16 everywhere on device (inputs, outputs); f32 only in PSUM.
    Measured rel err ~5.0e-3 vs the fp32 reference (gate 2e-2).
"""
import numpy as np
from contextlib import ExitStack

import ml_dtypes

import concourse.bacc as bacc
import concourse.mybir as mybir
import concourse.tile as tile
from concourse.bass_utils import run_bass_kernel_spmd

AF = mybir.ActivationFunctionType
F32 = mybir.dt.float32
BF16 = mybir.dt.bfloat16

B, T, D = 4, 4096, 4096
NCORES = 8
ROWS = (B * T) // NCORES        # 2048 rows per core
NQ = 4                          # quarters per core
QR = ROWS // NQ                 # 512 rows per quarter
DC = D // 128                   # 32 contraction chunks
GRP = 4                         # x chunks per DMA tile
NGRP = DC // GRP                # 8 groups
EPS = 1.1920929e-07


def build_nc(repeat=1, variant="full"):
    """variant: "full" = real kernel; "mm1"/"noout" are timing ablations."""
    nc = bacc.Bacc("TRN2", target_bir_lowering=False, debug=False,
                   num_devices=NCORES, enable_partition_id=False)

    # [q, g, p, j, r]: each x tile (4 chunks x 512 rows) is one fully
    # contiguous 4 KiB-per-partition DMA
    xq_in = nc.dram_tensor("xq", [NQ, NGRP, 128, GRP, QR], BF16,
                           kind="ExternalInput")
    wc_in = nc.dram_tensor("wc", [128, DC, 256], BF16, kind="ExternalInput")
    wdd_in = nc.dram_tensor("wdd", [128, DC, 64], BF16, kind="ExternalInput")
    qkw_in = nc.dram_tensor("qkw2", [128, 2, 128], BF16, kind="ExternalInput")
    # raw (pre-rms) w, transposed [c, q, im, r]: mm2 keeps qkw stationary
    # and streams g as one N=512 moving operand (2 MMs/quarter instead of
    # 8 short ones); the host epilogue untransposes
    out_main = nc.dram_tensor("out_main", [2, NQ, 128, QR], BF16,
                              kind="ExternalOutput")
    out_ddraw = nc.dram_tensor("out_ddraw", [128, ROWS], BF16,
                               kind="ExternalOutput")

    NQTOT = repeat * NQ

    with tile.TileContext(nc) as tc, ExitStack() as ctx:
        consts = ctx.enter_context(tc.tile_pool(name="consts", bufs=1))
        xpool = ctx.enter_context(tc.tile_pool(name="x", bufs=16))
        gpool = ctx.enter_context(tc.tile_pool(name="g", bufs=4))
        ddpool = ctx.enter_context(tc.tile_pool(name="dds", bufs=2))
        wpool = ctx.enter_context(tc.tile_pool(name="w", bufs=4))
        # PSUM (8 banks total): ppool = double-buffered hc accumulators
        # (2/quarter x 2 quarters); p2pool = shared 4-bank ring for the 4
        # dd col-tile accumulators + 2 mm2 outputs (6 allocs/quarter).
        # dd tiles are allocated at chunk-group 2 and first written at
        # group 3, by which time the previous quarter's ddcopy/mm2-copy
        # ACT reads have freed the aliased banks.
        ppool = ctx.enter_context(tc.tile_pool(name="ps", bufs=4, space="PSUM"))
        p2pool = ctx.enter_context(tc.tile_pool(name="ps2", bufs=4,
                                                space="PSUM"))

        wc_sb = consts.tile([128, DC, 256], BF16)
        wdd_sb = consts.tile([128, DC, 64], BF16)
        qkw_sb = consts.tile([128, 2, 128], BF16)

        # Prologue: weights interleaved with Q0's x tiles in consumption
        # order so the first matmuls don't wait on the full 2.6 MiB.
        q0_tiles = []
        for g in range(NGRP):
            nc.sync.dma_start(wc_sb[:, g * GRP:(g + 1) * GRP, :],
                              wc_in[:, g * GRP:(g + 1) * GRP, :])
            nc.sync.dma_start(wdd_sb[:, g * GRP:(g + 1) * GRP, :],
                              wdd_in[:, g * GRP:(g + 1) * GRP, :])
            xt = xpool.tile([128, GRP, QR], BF16, tag="xt")
            nc.sync.dma_start(xt[:], xq_in[0, g])
            q0_tiles.append(xt)
        nc.sync.dma_start(qkw_sb[:], qkw_in[:])

        def emit_mm1(Q, tiles, tail_prev, next_tiles_out):
            """mm1 for quarter Q; emits tail_prev() after group 1 and
            prefetch DMAs for Q+1 spread across the groups."""
            hc0 = ppool.tile([128, QR], F32, tag="ps")
            hc2 = ppool.tile([128, QR], F32, tag="ps")
            dd4t = []

            def dd_pairs(g):
                # 4-way col-tiling with 32-col (single col-group)
                # stationaries at (0,0/32/64/96) into 4 banks: 64-col
                # 2-way tiles never packed concurrently (measured), but
                # single-col-group tiles do (-7us/rep). Bank t holds
                # partitions [32t:32t+32]: [even-lo; even-hi; odd-lo;
                # odd-hi], so raw[0:64]+raw[64:128] on the host is still
                # [pre_total; post_total].
                xt = tiles[g]
                d0 = g * GRP
                for j in range(0, GRP, 2):
                    for t in range(4):
                        dj = j + t // 2
                        lo = (t % 2) * 32
                        nc.tensor.matmul(
                            dd4t[t][t * 32:(t + 1) * 32, :],
                            wdd_sb[:, d0 + dj, lo:lo + 32],
                            xt[:, dj, :], start=d0 + j == 0,
                            stop=d0 + j == DC - 2,
                            tile_position=(0, t * 32))

            for g in range(NGRP):
                xt = tiles[g]
                d0 = g * GRP
                # runs of GRP MMs per psum bank to limit bank cycling
                for j in range(GRP):
                    nc.tensor.matmul(hc0[:], wc_sb[:, d0 + j, 0:128],
                                     xt[:, j, :], start=d0 + j == 0,
                                     stop=d0 + j == DC - 1)
                for j in range(GRP):
                    nc.tensor.matmul(hc2[:], wc_sb[:, d0 + j, 128:256],
                                     xt[:, j, :], start=d0 + j == 0,
                                     stop=d0 + j == DC - 1)
                if g == 2:
                    # dd banks allocated late: they reuse ddcopy/mm2-freed
                    # banks of the 4-ring; first dd write is at g==3
                    for t in range(4):
                        dd4t.append(p2pool.tile([128, QR], F32,
                                                name=f"dd4_{t}", tag="ps2"))
                if g >= 3:
                    dd_pairs(g - 3)
                if g == 1 and tail_prev is not None:
                    tail_prev()
                if Q + 1 < NQTOT:
                    xt1 = xpool.tile([128, GRP, QR], BF16, tag="xt")
                    nc.sync.dma_start(xt1[:], xq_in[(Q + 1) % NQ, g])
                    next_tiles_out.append(xt1)
            for g in range(NGRP - 3, NGRP):
                dd_pairs(g)
            return hc0, hc2, dd4t

        def emit_head_tail(Q, hc0, hc2, dd4t):
            """ACT work right after mm1(Q): gelu + dd evacuation."""
            g_sb = gpool.tile([128, 2, QR], BF16, tag="g")
            nc.scalar.activation(g_sb[:, 0, :], hc0[:], AF.Gelu)
            nc.scalar.activation(g_sb[:, 1, :], hc2[:], AF.Gelu)
            dds = ddpool.tile([128, QR], BF16, tag="dds")
            # dd evacuation on the (otherwise idle) DVE so it overlaps the
            # gelus on ACT instead of queueing behind them
            for t in range(4):
                nc.vector.tensor_copy(dds[t * 32:(t + 1) * 32, :],
                                      dd4t[t][t * 32:(t + 1) * 32, :])
            return g_sb, dds

        def make_tail(Q, g_sb, dds):
            """mm2 + raw-w store for quarter Q (run during Q+1)."""
            q = Q % NQ

            def tail():
                if variant == "noout":
                    pass
                else:
                    nc.scalar.dma_start(out_ddraw[:, q * QR:(q + 1) * QR],
                                        dds[:])
                for c in range(2):
                    m2 = p2pool.tile([128, QR], F32, tag="ps2")
                    nc.tensor.matmul(m2[:], qkw_sb[:, c, :], g_sb[:, c, :],
                                     start=True, stop=True)
                    w = wpool.tile([128, QR], BF16, tag="wsb")
                    nc.scalar.activation(w[:], m2[:], AF.Copy)
                    if variant != "noout":
                        nc.scalar.dma_start(out_main[c, q], w[:])

            return tail

        tiles = q0_tiles
        tail_prev = None
        for Q in range(NQTOT):
            next_tiles = []
            hc0, hc2, dd4t = emit_mm1(Q, tiles, tail_prev, next_tiles)
            g_sb, dds = emit_head_tail(Q, hc0, hc2, dd4t)
            tail_prev = None if variant == "mm1" else make_tail(Q, g_sb, dds)
            tiles = next_tiles
        if tail_prev is not None:
            tail_prev()

    nc.compile()
    return nc


def host_prep(query_vec, dw1, qkw, dd, norm_scale):
    """Per-core input maps, all bf16."""
    x = np.ascontiguousarray(query_vec.reshape(B * T, D)).astype(
        ml_dtypes.bfloat16)

    wsel = dw1[:, 0][:, [0, 2], :].reshape(D, 256)
    wc_h = np.ascontiguousarray(
        wsel.reshape(DC, 128, 256).transpose(1, 0, 2)).astype(
        ml_dtypes.bfloat16)                                    # [128, DC, 256]
    ddsel = np.concatenate([dd[:, 0, 0:32], dd[:, 0, 64:96]], axis=1)
    wdd_h = np.ascontiguousarray(
        ddsel.reshape(DC, 128, 64).transpose(1, 0, 2)).astype(
        ml_dtypes.bfloat16)                                    # [128, DC, 64]
    qkw2 = np.ascontiguousarray(
        qkw[0, [0, 2]].reshape(2, 128, 128).transpose(1, 0, 2)
    ).astype(ml_dtypes.bfloat16)                               # [128, 2, 128]

    in_maps = []
    for c in range(NCORES):
        xc = x[c * ROWS:(c + 1) * ROWS]                        # [2048, 4096]
        xh = np.ascontiguousarray(
            xc.reshape(NQ, QR, NGRP, GRP, 128).transpose(0, 2, 4, 3, 1))
        in_maps.append({"xq": xh, "wc": wc_h, "wdd": wdd_h, "qkw2": qkw2})
    return in_maps


def host_post(results, norm_scale):
    """rms-normalize raw w, finish dd (sum halves + tanh), assemble."""
    s = float(np.asarray(norm_scale).reshape(-1)[0])
    outs = []
    for c in range(NCORES):
        om = np.asarray(results[c]["out_main"], dtype=np.float32)
        # [c, q, im, r] -> [q, r, c, im] -> [rows, c, i, m]
        w = om.transpose(1, 3, 0, 2).reshape(ROWS, 2, 4, 32)
        rms = np.sqrt(np.mean(w * w, axis=-1, keepdims=True) + EPS)
        w = w / rms
        w[:, :, 2:4] *= s
        w = w.reshape(ROWS, 2, 128)
        ddraw = np.asarray(results[c]["out_ddraw"], dtype=np.float32)
        ddv = np.tanh(ddraw[0:64] + ddraw[64:128]).T           # [2048, 64]
        outs.append(np.concatenate(
            [w[:, 0], ddv[:, 0:32], w[:, 1], ddv[:, 32:64]], axis=1))
    return np.concatenate(outs, axis=0).reshape(B, T, 320)


_NC_CACHE = {}


def get_nc():
    if "nc" not in _NC_CACHE:
        _NC_CACHE["nc"] = build_nc()
    return _NC_CACHE["nc"]


def _run_device(nc, in_maps):
    return run_bass_kernel_spmd(nc, in_maps, list(range(NCORES))).results


def _run_subprocess(query_vec, dw1, qkw, dd, norm_scale):
    """Fresh-process fallback: a crashed/wedged device state lives in the
    axon client; a clean process (with core reset) usually recovers."""
    import os
    import subprocess
    import sys
    import tempfile
    d = tempfile.mkdtemp(prefix="dwp_kernel_")
    np.save(os.path.join(d, "query_vec.npy"), query_vec)
    np.save(os.path.join(d, "dw1.npy"), dw1)
    np.save(os.path.join(d, "qkw.npy"), qkw)
    np.save(os.path.join(d, "dd.npy"), dd)
    np.save(os.path.join(d, "norm_scale.npy"), norm_scale)
    prog = (
        "import numpy as np, importlib.util, sys\n"
        f"spec = importlib.util.spec_from_file_location('dwp_kernel', {__file__!r})\n"
        "m = importlib.util.module_from_spec(spec); spec.loader.exec_module(m)\n"
        f"d = {d!r}\n"
        "ins = {k: np.load(d + '/' + k + '.npy') for k in"
        " ('query_vec', 'dw1', 'qkw', 'dd', 'norm_scale')}\n"
        "out = m.kernel(_allow_subprocess=False, **ins)\n"
        "np.save(d + '/out.npy', out)\n"
    )
    env = dict(os.environ)
    env["NEURON_RT_RESET_CORES"] = "1"
    subprocess.run([sys.executable, "-c", prog], check=True, env=env,
                   timeout=1800)
    return np.load(os.path.join(d, "out.npy"))


def kernel(query_vec, dw1, qkw, dd, norm_scale, _allow_subprocess=True):
    nc = get_nc()
    in_maps = host_prep(query_vec, dw1, qkw, dd, norm_scale)
    try:
        res = _run_device(nc, in_maps)
    except Exception:
        if not _allow_subprocess:
            raise
        try:
            res = _run_device(nc, in_maps)       # in-process retry
        except Exception:
            return _run_subprocess(query_vec, dw1, qkw, dd, norm_scale)
    return host_post(res, norm_scale)
